# revision 25
# baseline (speedup 1.0000x reference)
"""AttentionPooling (segment softmax-pool) TRN2 kernel, 8-core SPMD.

Self-contained: kernel(**inputs) -> np.ndarray [16384, 128] f32.

Math (shift-invariance of softmax; logits are O(1) so exp can't overflow):
  e_i   = exp(tanh(x_i @ W1 + b1) @ W2 + b2)
  out_g = (sum_{i in g} e_i x_i) / (sum_{i in g} e_i)

Sharding: graphs are split into 8 contiguous ranges with ~equal node counts
(each graph's nodes land on one core); each core computes its own rows of the
output; host concatenates.

Device algorithm per core (see build_program): x is streamed once in two bf16
layouts (natural tiles for the pooling matmul, transposed for the MLP matmul).
A 3-deep software pipeline keeps the PE gapless (high p-state): step s runs
ht-matmuls for chunk s, logit matmuls for chunk s-1, pooling matmuls for
chunk s-2.  The per-window slot-expansion (masked e) is built with two
chunk-level DVE tensor_tensor ops (is_equal on a broadcast seg image, then
multiply by broadcast e) instead of per-tile tensor_scalar ops.  A final
static indicator matmul re-bins window-slots to segments, then DVE
reciprocal+scale normalizes.
"""

import math

import numpy as np
import ml_dtypes

BF16 = ml_dtypes.bfloat16

N_CORES = 8
N_GRAPHS = 16384
H = 128
TILE = 128
TPC = 32             # tiles per window
CHUNK = TILE * TPC   # 4096 rows
SLOTS = 64           # max segments per window
NW_STITCH = 8        # stitch window span (static)
NCHK = NW_STITCH * SLOTS // 128
MSLOT = 48           # active slot width (real data max is 36/window)
PAD_SEG = 9999.0

LAST_EXEC_NS = None
_PROGRAM_CACHE = {}


# ---------------------------------------------------------------- host prep
def _preprocess(x, batch, W1, b1, W2, b2, n_graphs):
    N = x.shape[0]
    counts = np.bincount(batch, minlength=n_graphs)
    cum = np.zeros(n_graphs + 1, dtype=np.int64)
    np.cumsum(counts, out=cum[1:])

    gsplit = [0]
    for c in range(1, N_CORES):
        t = round(c * N / N_CORES)
        g = int(np.searchsorted(cum, t))
        if g > 0 and abs(cum[g - 1] - t) <= abs(cum[g] - t):
            g -= 1
        g = max(g, gsplit[-1] + 1)
        gsplit.append(min(g, n_graphs - (N_CORES - c)))
    gsplit.append(n_graphs)
    gsplit = np.array(gsplit, dtype=np.int64)

    Mc = [int(cum[gsplit[c + 1]] - cum[gsplit[c]]) for c in range(N_CORES)]
    Gc = [int(gsplit[c + 1] - gsplit[c]) for c in range(N_CORES)]
    NWIN = max(NW_STITCH, math.ceil(max(Mc) / CHUNK))
    if NWIN % 2:
        NWIN += 1
    M_pad = NWIN * CHUNK
    NGRP = math.ceil(max(Gc) / 128)

    x = np.asarray(x, dtype=np.float32)
    batch = np.asarray(batch)

    cores = []
    minw = np.full((N_CORES, NGRP), 10 ** 9, dtype=np.int64)
    maxw = np.full((N_CORES, NGRP), -1, dtype=np.int64)
    for c in range(N_CORES):
        nlo = int(cum[gsplit[c]])
        nhi = int(cum[gsplit[c + 1]])
        m = Mc[c]
        bl = batch[nlo:nhi].astype(np.int64) - gsplit[c]
        wfs = np.zeros(NWIN, dtype=np.int64)
        for w in range(NWIN):
            wfs[w] = bl[w * CHUNK] if w * CHUNK < m else Gc[c]
        slots = bl - wfs[np.arange(m) // CHUNK]
        assert slots.min() >= 0 and slots.max() < MSLOT, (
            f"core {c}: window slot range {slots.min()}..{slots.max()}")

        seg = np.full(M_pad, PAD_SEG, dtype=np.float32)
        seg[:m] = slots.astype(np.float32)
        seg_img = np.ascontiguousarray(seg.reshape(-1, TILE).T.astype(BF16))

        nT = M_pad // TILE
        xn = np.zeros((M_pad, H + 1), dtype=BF16)
        xn[:m, :H] = x[nlo:nhi]
        xn[:, H] = 1.0
        xn_img = np.ascontiguousarray(
            xn.reshape(nT, TILE, H + 1).transpose(1, 0, 2).reshape(TILE, nT * (H + 1)))

        xt = np.zeros((M_pad, H), dtype=BF16)
        xt[:m] = x[nlo:nhi]
        xt_img = np.ascontiguousarray(xt.T)

        lo_g = cum[gsplit[c]:gsplit[c + 1]] - nlo
        hi_g = cum[gsplit[c] + 1:gsplit[c + 1] + 1] - nlo
        nonempty = hi_g > lo_g
        wlo_g = np.where(nonempty, lo_g // CHUNK, 0)
        whi_g = np.where(nonempty, np.maximum(hi_g - 1, 0) // CHUNK, 0)
        for Gi in range(NGRP):
            a, b = Gi * 128, min(Gi * 128 + 128, Gc[c])
            if a >= Gc[c]:
                continue
            ne = nonempty[a:b]
            if ne.any():
                minw[c, Gi] = wlo_g[a:b][ne].min()
                maxw[c, Gi] = whi_g[a:b][ne].max()
        cores.append(dict(m=m, gc=Gc[c], wfs=wfs, seg_img=seg_img,
                          xn_img=xn_img, xt_img=xt_img, nonempty=nonempty,
                          wlo_g=wlo_g, whi_g=whi_g))

    wlo_shared = []
    for Gi in range(NGRP):
        mn = int(minw[:, Gi].min())
        if mn >= 10 ** 9:
            mn = 0
        mn -= mn % 2
        mn = max(0, min(mn, NWIN - NW_STITCH))
        wlo_shared.append(mn)
        mx = int(maxw[:, Gi].max())
        assert mx < 0 or mx - mn + 1 <= NW_STITCH, (
            f"group {Gi}: window span {mn}..{mx} exceeds {NW_STITCH}")

    for c in range(N_CORES):
        d = cores[c]
        ind = np.zeros((128, NGRP * NCHK * 128), dtype=np.float32)
        for g in range(d["gc"]):
            if not d["nonempty"][g]:
                continue
            Gi = g // 128
            base_ws = wlo_shared[Gi] * SLOTS
            for w in range(int(d["wlo_g"][g]), int(d["whi_g"][g]) + 1):
                s = int(g - d["wfs"][w])
                wsl = w * SLOTS + s - base_ws
                assert 0 <= wsl < NW_STITCH * SLOTS
                ind[wsl % 128, (Gi * NCHK + wsl // 128) * 128 + (g - Gi * 128)] = 1.0
        d["ind_img"] = np.ascontiguousarray(ind.astype(BF16))

    shared = dict(
        NWIN=NWIN, M_pad=M_pad, NGRP=NGRP, wlo_shared=wlo_shared,
        gsplit=gsplit, counts=counts,
        iota=np.ascontiguousarray(
            np.broadcast_to(np.arange(MSLOT, dtype=BF16), (128, MSLOT))),
        w1b=np.ascontiguousarray(np.asarray(W1).astype(BF16)),
        w2b=np.ascontiguousarray(np.asarray(W2).astype(BF16)),
        b1c=np.ascontiguousarray(np.asarray(b1).reshape(H, 1).astype(np.float32)),
        b2c=np.full((128, 1), np.asarray(b2).reshape(-1)[0], dtype=np.float32),
    )
    return shared, cores


# ---------------------------------------------------------------- program
def _build_program(NWIN, NGRP, wlo_shared):
    from contextlib import ExitStack
    import concourse.bacc as bacc
    import concourse.tile as tile
    from concourse import mybir

    M_pad = NWIN * CHUNK
    nT = M_pad // TILE
    NWCOL = NWIN * SLOTS // 128

    f32 = mybir.dt.float32
    bf16 = mybir.dt.bfloat16
    AF = mybir.ActivationFunctionType
    ALU = mybir.AluOpType

    nc = bacc.Bacc("TRN2", target_bir_lowering=False, debug=False,
                   enable_asserts=False, num_devices=N_CORES)
    xt_ap = nc.dram_tensor("xT", [128, M_pad], bf16, kind="ExternalInput").ap()
    xn_ap = nc.dram_tensor("xn", [128, nT * (H + 1)], bf16, kind="ExternalInput").ap()
    seg_ap = nc.dram_tensor("seg", [128, nT], bf16, kind="ExternalInput").ap()
    iota_ap = nc.dram_tensor("iota", [128, MSLOT], bf16, kind="ExternalInput").ap()
    w1_ap = nc.dram_tensor("w1b", [128, H], bf16, kind="ExternalInput").ap()
    w2_ap = nc.dram_tensor("w2b", [128, 1], bf16, kind="ExternalInput").ap()
    b1_ap = nc.dram_tensor("b1c", [128, 1], f32, kind="ExternalInput").ap()
    b2_ap = nc.dram_tensor("b2c", [128, 1], f32, kind="ExternalInput").ap()
    ind_ap = nc.dram_tensor("ind", [128, NGRP * NCHK * 128], bf16,
                            kind="ExternalInput").ap()
    out_ap = nc.dram_tensor("out", [NGRP * 128, H], f32, kind="ExternalOutput").ap()

    with tile.TileContext(nc) as tc, ExitStack() as ctx:
        consts = ctx.enter_context(tc.tile_pool(name="consts", bufs=1))
        xt_pool = ctx.enter_context(tc.tile_pool(name="xt", bufs=4))
        xn_pool = ctx.enter_context(tc.tile_pool(name="xnp", bufs=9))
        seg_pool = ctx.enter_context(tc.tile_pool(name="segp", bufs=5))
        h_pool = ctx.enter_context(tc.tile_pool(name="hp", bufs=12))
        e_pool = ctx.enter_context(tc.tile_pool(name="ep", bufs=3))
        eq_pool = ctx.enter_context(tc.tile_pool(name="eqp", bufs=3))
        me_poolA = ctx.enter_context(tc.tile_pool(name="mepA", bufs=3))
        me_poolB = ctx.enter_context(tc.tile_pool(name="mepB", bufs=3))
        wres_pool = ctx.enter_context(tc.tile_pool(name="wres", bufs=1))
        r_pool = ctx.enter_context(tc.tile_pool(name="rp", bufs=2))
        ob_pool = ctx.enter_context(tc.tile_pool(name="obp", bufs=2))
        ht_psum = ctx.enter_context(tc.tile_pool(name="htps", bufs=3, space="PSUM"))
        lg_psum = ctx.enter_context(tc.tile_pool(name="lgps", bufs=1, space="PSUM"))
        pl_psum = ctx.enter_context(tc.tile_pool(name="plps", bufs=1, space="PSUM"))

        iota_t = consts.tile([128, MSLOT], bf16, tag="iota")
        nc.sync.dma_start(iota_t[:], iota_ap[:])
        w1_t = consts.tile([128, H], bf16, tag="w1")
        nc.sync.dma_start(w1_t[:], w1_ap[:])
        w2_t = consts.tile([128, 1], bf16, tag="w2")
        nc.sync.dma_start(w2_t[:], w2_ap[:])
        b1_t = consts.tile([128, 1], f32, tag="b1")
        nc.sync.dma_start(b1_t[:], b1_ap[:])
        b2_t = consts.tile([128, 1], f32, tag="b2")
        nc.sync.dma_start(b2_t[:], b2_ap[:])
        # all indicator tiles live in SBUF for the whole kernel; the DMA is
        # issued after the first chunks' so it doesn't delay the pipeline
        # start (first stitch consumer is ~10 steps in)
        ind_t = consts.tile([128, NGRP * NCHK * 128], bf16, tag="ind")
        wres_cols = [wres_pool.tile([128, H + 1], bf16, name=f"wres{i}",
                                    tag=f"wres{i}")
                     for i in range(NWCOL)]
        for i in range(NWCOL):
            nc.vector.memset(wres_cols[i][:], 0.0)

        # emit group Gi's stitch right after its last window is flushed
        ready_groups = {}
        for Gi in range(NGRP):
            ready_groups.setdefault(wlo_shared[Gi] + NW_STITCH - 1, []).append(Gi)

        def emit_stitch(Gi):
            st = lg_psum.tile([128, H + 1], f32, tag="lg")
            for k in range(NCHK):
                wc = wlo_shared[Gi] // 2 + k
                nc.tensor.matmul(
                    st[:], lhsT=ind_t[:, (Gi * NCHK + k) * 128:(Gi * NCHK + k + 1) * 128],
                    rhs=wres_cols[wc][:],
                    start=(k == 0), stop=(k == NCHK - 1))
            r = r_pool.tile([128, 1], f32)
            nc.vector.reciprocal(r[:], st[:, H:H + 1])
            ob = ob_pool.tile([128, H], f32)
            nc.vector.tensor_scalar(ob[:], st[:, 0:H], r[:, 0:1], None,
                                    op0=ALU.mult)
            nc.sync.dma_start(out_ap[Gi * 128:(Gi + 1) * 128, :], ob[:])

        # ---- pipeline stages -------------------------------------------
        # step s runs: ht q0,q1 (s) | logits+exp+masked-e (s-1) |
        #              ht q2,q3 (s) | pooling+stitch (s-2)
        # The logit block between the ht halves gives the Act engine time to
        # drain tanh(q0) before the PE reuses its PSUM buffer for q2.
        def s_mlp_a(c):
            """DMA chunk c; ht matmuls+tanh for cols 0:2048; chunk-level eq."""
            xt = xt_pool.tile([128, CHUNK], bf16, name=f"xt{c}", tag="xt")
            nc.sync.dma_start(xt[:], xt_ap[:, c * CHUNK:(c + 1) * CHUNK])
            xn = xn_pool.tile([128, TPC * (H + 1)], bf16, name=f"xn{c}", tag="xn")
            nc.sync.dma_start(
                xn[:], xn_ap[:, c * TPC * (H + 1):(c + 1) * TPC * (H + 1)])
            sg = seg_pool.tile([128, TPC], bf16, name=f"sg{c}", tag="sg")
            nc.sync.dma_start(sg[:], seg_ap[:, c * TPC:(c + 1) * TPC])

            st = dict(xt=xt, xn=xn, hqs=[])
            for quarter in range(2):
                _ht_quarter(c, st, quarter)
            # chunk-level eq: eq[p, t, s] = (sg[p, t] == iota[s])
            eq = eq_pool.tile([128, TPC, MSLOT], bf16, name=f"eq{c}", tag="eq")
            nc.vector.tensor_tensor(
                eq[:], sg[:, :, None].broadcast_to([128, TPC, MSLOT]),
                iota_t[:, None, :].broadcast_to([128, TPC, MSLOT]),
                op=ALU.is_equal)
            st["eq"] = eq
            return st

        def _ht_quarter(c, st, quarter):
            ht = ht_psum.tile([128, 1024], f32, name=f"ht{c}_{quarter}",
                              tag="ht")
            for j in range(2):
                lo = quarter * 1024 + j * 512
                nc.tensor.matmul(ht[:, j * 512:(j + 1) * 512], lhsT=w1_t[:],
                                 rhs=st["xt"][:, lo:lo + 512],
                                 start=True, stop=True)
            hq = h_pool.tile([128, 1024], bf16, name=f"hq{c}_{quarter}",
                             tag="hq")
            nc.scalar.activation(hq[:], ht[:], AF.Tanh, bias=b1_t[:, 0:1])
            st["hqs"].append(hq)

        def s_mlp_b(c, st):
            for quarter in range(2, 4):
                _ht_quarter(c, st, quarter)

        def s_lg(c, st):
            """32 logit matmuls for chunk c; exp."""
            lg = lg_psum.tile([128, TPC], f32, name=f"lg{c}", tag="lg")
            for t in range(TPC):
                hq = st["hqs"][t // 8]
                nc.tensor.matmul(lg[:, t:t + 1],
                                 lhsT=hq[:, (t % 8) * 128:(t % 8 + 1) * 128],
                                 rhs=w2_t[:], start=True, stop=True)
            ee = e_pool.tile([128, TPC], bf16, name=f"ee{c}", tag="ee")
            nc.scalar.activation(ee[:], lg[:], AF.Exp, bias=b2_t[:, 0:1])
            st["ee"] = ee

        def s_memult(c, st):
            """Chunk-level masked-e.  me pools alternate per chunk and the
            pooling depth is even, so a pooling block's pool-level read
            threshold lands on a MULTIPLY finished two steps ago (the
            scheduler serializes pool readers against the latest writer in
            final order, whatever the emission order)."""
            me_pool = me_poolA if c % 2 == 0 else me_poolB
            me = me_pool.tile([128, TPC, MSLOT], bf16, name=f"me{c}", tag="me")
            nc.vector.tensor_tensor(
                me[:], st["eq"][:],
                st["ee"][:, :, None].broadcast_to([128, TPC, MSLOT]),
                op=ALU.mult)
            st["me"] = me

        def s_pool_mm(c, st):
            """32 pooling matmuls for chunk c (PE only)."""
            strip = 64 * (c % 2)
            pl = pl_psum.tile([128, H + 1], f32, name=f"pl{c}", tag="pl")
            xn, me = st["xn"], st["me"]
            for t in range(TPC):
                nc.tensor.matmul(
                    pl[strip:strip + MSLOT, :], lhsT=me[:, t, :],
                    rhs=xn[:, t * (H + 1):(t + 1) * (H + 1)],
                    start=(t == 0), stop=(t == TPC - 1),
                    tile_position=(0, strip))
            st["pl"] = pl
            st["strip"] = strip

        def s_pool_flush(c, st):
            """Flush chunk c's window strip to wres; run stitches that became
            ready LAST step (their wres inputs were flushed a step ago, so
            the stitch matmuls never wait on this step's CAST)."""
            nc.scalar.activation(
                wres_cols[c // 2][st["strip"]:st["strip"] + MSLOT, :],
                st["pl"][st["strip"]:st["strip"] + MSLOT, :], AF.Copy)
            for Gi in ready_groups.get(c - 1, ()):
                emit_stitch(Gi)

        # Step order matters: the tile framework serializes a pool's readers
        # against the latest writer EMITTED so far, so the pooling block for
        # chunk c-3 is emitted before this step's masked-e MULTIPLY, and the
        # logit block sits between the two ht halves so tanh(q0) completes
        # before its PSUM buffer is reused.
        # flush (CAST + stitches) leads the NEXT step so neither the Act tail
        # nor the PE stitch matmuls ever wait on freshly-emitted producers
        ps = [None] * 7
        for c in range(NWIN):
            if ps[6] is not None:
                s_pool_flush(c - 7, ps[6])
            if ps[0] is not None:
                s_lg(c - 1, ps[0])
                s_memult(c - 1, ps[0])
            cur = s_mlp_a(c)
            if c == 3:
                nc.sync.dma_start(ind_t[:], ind_ap[:])
            s_mlp_b(c, cur)
            if ps[5] is not None:
                s_pool_mm(c - 6, ps[5])
            ps = [cur] + ps[:6]
        s_lg(NWIN - 1, ps[0])
        s_memult(NWIN - 1, ps[0])
        if ps[6] is not None:
            s_pool_flush(NWIN - 7, ps[6])
        for k in range(5, -1, -1):
            s_pool_mm(NWIN - 1 - k, ps[k])
            s_pool_flush(NWIN - 1 - k, ps[k])
        for Gi in ready_groups.get(NWIN - 1, ()):
            emit_stitch(Gi)

    nc.compile()
    return nc


def kernel(x, batch, W1, b1, W2, b2):
    global LAST_EXEC_NS
    import os
    from concourse.bass_utils import run_bass_kernel_spmd

    x = np.asarray(x)
    batch = np.asarray(batch)
    shared, cores = _preprocess(x, batch, W1, b1, W2, b2, N_GRAPHS)

    key = (shared["NWIN"], shared["NGRP"], tuple(shared["wlo_shared"]))
    nc = _PROGRAM_CACHE.get(key)
    if nc is None:
        nc = _build_program(shared["NWIN"], shared["NGRP"], shared["wlo_shared"])
        _PROGRAM_CACHE[key] = nc

    in_maps = []
    for d in cores:
        in_maps.append({
            "xT": d["xt_img"], "xn": d["xn_img"], "seg": d["seg_img"],
            "iota": shared["iota"], "w1b": shared["w1b"], "w2b": shared["w2b"],
            "b1c": shared["b1c"], "b2c": shared["b2c"], "ind": d["ind_img"],
        })
    trace = os.environ.get("ATTNPOOL_TRACE", "0") == "1"
    res = run_bass_kernel_spmd(nc, in_maps, core_ids=list(range(N_CORES)),
                               trace=trace)
    if res.exec_time_ns is not None:
        LAST_EXEC_NS = res.exec_time_ns

    out = np.zeros((N_GRAPHS, H), dtype=np.float32)
    gsplit = shared["gsplit"]
    for c, d in enumerate(cores):
        out[gsplit[c]:gsplit[c + 1]] = res.results[c]["out"][:d["gc"]]
    out[shared["counts"] == 0] = 0.0
    return out


# revision 26
# speedup vs baseline: 1.0298x; 1.0298x over previous
"""AttentionPooling (segment softmax-pool) TRN2 kernel, 8-core SPMD.

Self-contained: kernel(**inputs) -> np.ndarray [16384, 128] f32.

Math (shift-invariance of softmax; logits are O(1) so exp can't overflow):
  e_i   = exp(tanh(x_i @ W1 + b1) @ W2 + b2)
  out_g = (sum_{i in g} e_i x_i) / (sum_{i in g} e_i)

Sharding: graphs are split into 8 contiguous ranges with ~equal node counts
(each graph's nodes land on one core); each core computes its own rows of the
output; host concatenates.

Device algorithm per core (see build_program): x is streamed once in two bf16
layouts (natural tiles for the pooling matmul, transposed for the MLP matmul).
A 3-deep software pipeline keeps the PE gapless (high p-state): step s runs
ht-matmuls for chunk s, logit matmuls for chunk s-1, pooling matmuls for
chunk s-2.  The per-window slot-expansion (masked e) is built with two
chunk-level DVE tensor_tensor ops (is_equal on a broadcast seg image, then
multiply by broadcast e) instead of per-tile tensor_scalar ops.  A final
static indicator matmul re-bins window-slots to segments, then DVE
reciprocal+scale normalizes.
"""

import math

import numpy as np
import ml_dtypes

BF16 = ml_dtypes.bfloat16

N_CORES = 8
N_GRAPHS = 16384
H = 128
TILE = 128
TPC = 32             # tiles per window
CHUNK = TILE * TPC   # 4096 rows
SLOTS = 64           # max segments per window
NW_STITCH = 8        # stitch window span (static)
NCHK = NW_STITCH * SLOTS // 128
MSLOT = 48           # active slot width (real data max is 36/window)
PAD_SEG = 9999.0

LAST_EXEC_NS = None
_PROGRAM_CACHE = {}


# ---------------------------------------------------------------- host prep
def _preprocess(x, batch, W1, b1, W2, b2, n_graphs):
    N = x.shape[0]
    counts = np.bincount(batch, minlength=n_graphs)
    cum = np.zeros(n_graphs + 1, dtype=np.int64)
    np.cumsum(counts, out=cum[1:])

    gsplit = [0]
    for c in range(1, N_CORES):
        t = round(c * N / N_CORES)
        g = int(np.searchsorted(cum, t))
        if g > 0 and abs(cum[g - 1] - t) <= abs(cum[g] - t):
            g -= 1
        g = max(g, gsplit[-1] + 1)
        gsplit.append(min(g, n_graphs - (N_CORES - c)))
    gsplit.append(n_graphs)
    gsplit = np.array(gsplit, dtype=np.int64)

    Mc = [int(cum[gsplit[c + 1]] - cum[gsplit[c]]) for c in range(N_CORES)]
    Gc = [int(gsplit[c + 1] - gsplit[c]) for c in range(N_CORES)]
    NWIN = max(NW_STITCH, math.ceil(max(Mc) / CHUNK))
    if NWIN % 2:
        NWIN += 1
    M_pad = NWIN * CHUNK
    NGRP = math.ceil(max(Gc) / 128)

    x = np.asarray(x, dtype=np.float32)
    batch = np.asarray(batch)

    cores = []
    minw = np.full((N_CORES, NGRP), 10 ** 9, dtype=np.int64)
    maxw = np.full((N_CORES, NGRP), -1, dtype=np.int64)
    for c in range(N_CORES):
        nlo = int(cum[gsplit[c]])
        nhi = int(cum[gsplit[c + 1]])
        m = Mc[c]
        bl = batch[nlo:nhi].astype(np.int64) - gsplit[c]
        wfs = np.zeros(NWIN, dtype=np.int64)
        for w in range(NWIN):
            wfs[w] = bl[w * CHUNK] if w * CHUNK < m else Gc[c]
        slots = bl - wfs[np.arange(m) // CHUNK]
        assert slots.min() >= 0 and slots.max() < MSLOT, (
            f"core {c}: window slot range {slots.min()}..{slots.max()}")

        seg = np.full(M_pad, PAD_SEG, dtype=np.float32)
        seg[:m] = slots.astype(np.float32)
        seg_img = np.ascontiguousarray(seg.reshape(-1, TILE).T.astype(BF16))

        nT = M_pad // TILE
        xn = np.zeros((M_pad, H + 1), dtype=BF16)
        xn[:m, :H] = x[nlo:nhi]
        xn[:, H] = 1.0
        xn_img = np.ascontiguousarray(
            xn.reshape(nT, TILE, H + 1).transpose(1, 0, 2).reshape(TILE, nT * (H + 1)))

        xt = np.zeros((M_pad, H), dtype=BF16)
        xt[:m] = x[nlo:nhi]
        xt_img = np.ascontiguousarray(xt.T)

        lo_g = cum[gsplit[c]:gsplit[c + 1]] - nlo
        hi_g = cum[gsplit[c] + 1:gsplit[c + 1] + 1] - nlo
        nonempty = hi_g > lo_g
        wlo_g = np.where(nonempty, lo_g // CHUNK, 0)
        whi_g = np.where(nonempty, np.maximum(hi_g - 1, 0) // CHUNK, 0)
        for Gi in range(NGRP):
            a, b = Gi * 128, min(Gi * 128 + 128, Gc[c])
            if a >= Gc[c]:
                continue
            ne = nonempty[a:b]
            if ne.any():
                minw[c, Gi] = wlo_g[a:b][ne].min()
                maxw[c, Gi] = whi_g[a:b][ne].max()
        cores.append(dict(m=m, gc=Gc[c], wfs=wfs, seg_img=seg_img,
                          xn_img=xn_img, xt_img=xt_img, nonempty=nonempty,
                          wlo_g=wlo_g, whi_g=whi_g))

    wlo_shared = []
    for Gi in range(NGRP):
        mn = int(minw[:, Gi].min())
        if mn >= 10 ** 9:
            mn = 0
        mn -= mn % 2
        mn = max(0, min(mn, NWIN - NW_STITCH))
        wlo_shared.append(mn)
        mx = int(maxw[:, Gi].max())
        assert mx < 0 or mx - mn + 1 <= NW_STITCH, (
            f"group {Gi}: window span {mn}..{mx} exceeds {NW_STITCH}")

    for c in range(N_CORES):
        d = cores[c]
        ind = np.zeros((128, NGRP * NCHK * 128), dtype=np.float32)
        for g in range(d["gc"]):
            if not d["nonempty"][g]:
                continue
            Gi = g // 128
            base_ws = wlo_shared[Gi] * SLOTS
            for w in range(int(d["wlo_g"][g]), int(d["whi_g"][g]) + 1):
                s = int(g - d["wfs"][w])
                wsl = w * SLOTS + s - base_ws
                assert 0 <= wsl < NW_STITCH * SLOTS
                ind[wsl % 128, (Gi * NCHK + wsl // 128) * 128 + (g - Gi * 128)] = 1.0
        d["ind_img"] = np.ascontiguousarray(ind.astype(BF16))

    shared = dict(
        NWIN=NWIN, M_pad=M_pad, NGRP=NGRP, wlo_shared=wlo_shared,
        gsplit=gsplit, counts=counts,
        iota=np.ascontiguousarray(
            np.broadcast_to(np.arange(MSLOT, dtype=BF16), (128, MSLOT))),
        w1b=np.ascontiguousarray(np.asarray(W1).astype(BF16)),
        w2b=np.ascontiguousarray(np.asarray(W2).astype(BF16)),
        b1c=np.ascontiguousarray(np.asarray(b1).reshape(H, 1).astype(np.float32)),
        b2c=np.full((128, 1), np.asarray(b2).reshape(-1)[0], dtype=np.float32),
    )
    return shared, cores


# ---------------------------------------------------------------- program
def _build_program(NWIN, NGRP, wlo_shared):
    from contextlib import ExitStack
    import concourse.bacc as bacc
    import concourse.tile as tile
    from concourse import mybir

    M_pad = NWIN * CHUNK
    nT = M_pad // TILE
    NWCOL = NWIN * SLOTS // 128

    f32 = mybir.dt.float32
    bf16 = mybir.dt.bfloat16
    AF = mybir.ActivationFunctionType
    ALU = mybir.AluOpType

    nc = bacc.Bacc("TRN2", target_bir_lowering=False, debug=False,
                   enable_asserts=False, num_devices=N_CORES)
    xt_ap = nc.dram_tensor("xT", [128, M_pad], bf16, kind="ExternalInput").ap()
    xn_ap = nc.dram_tensor("xn", [128, nT * (H + 1)], bf16, kind="ExternalInput").ap()
    seg_ap = nc.dram_tensor("seg", [128, nT], bf16, kind="ExternalInput").ap()
    iota_ap = nc.dram_tensor("iota", [128, MSLOT], bf16, kind="ExternalInput").ap()
    w1_ap = nc.dram_tensor("w1b", [128, H], bf16, kind="ExternalInput").ap()
    w2_ap = nc.dram_tensor("w2b", [128, 1], bf16, kind="ExternalInput").ap()
    b1_ap = nc.dram_tensor("b1c", [128, 1], f32, kind="ExternalInput").ap()
    b2_ap = nc.dram_tensor("b2c", [128, 1], f32, kind="ExternalInput").ap()
    ind_ap = nc.dram_tensor("ind", [128, NGRP * NCHK * 128], bf16,
                            kind="ExternalInput").ap()
    out_ap = nc.dram_tensor("out", [NGRP * 128, H], f32, kind="ExternalOutput").ap()

    with tile.TileContext(nc) as tc, ExitStack() as ctx:
        consts = ctx.enter_context(tc.tile_pool(name="consts", bufs=1))
        xt_pool = ctx.enter_context(tc.tile_pool(name="xt", bufs=4))
        xn_pool = ctx.enter_context(tc.tile_pool(name="xnp", bufs=8))
        seg_pool = ctx.enter_context(tc.tile_pool(name="segp", bufs=5))
        h_pool = ctx.enter_context(tc.tile_pool(name="hp", bufs=12))
        e_pool = ctx.enter_context(tc.tile_pool(name="ep", bufs=3))
        eq_pool = ctx.enter_context(tc.tile_pool(name="eqp", bufs=3))
        me_pool = ctx.enter_context(tc.tile_pool(name="mep", bufs=6))
        wres_pool = ctx.enter_context(tc.tile_pool(name="wres", bufs=1))
        r_pool = ctx.enter_context(tc.tile_pool(name="rp", bufs=2))
        ob_pool = ctx.enter_context(tc.tile_pool(name="obp", bufs=2))
        ht_psum = ctx.enter_context(tc.tile_pool(name="htps", bufs=3, space="PSUM"))
        lg_psum = ctx.enter_context(tc.tile_pool(name="lgps", bufs=1, space="PSUM"))
        pl_psum = ctx.enter_context(tc.tile_pool(name="plps", bufs=1, space="PSUM"))

        iota_t = consts.tile([128, MSLOT], bf16, tag="iota")
        nc.sync.dma_start(iota_t[:], iota_ap[:])
        w1_t = consts.tile([128, H], bf16, tag="w1")
        nc.sync.dma_start(w1_t[:], w1_ap[:])
        w2_t = consts.tile([128, 1], bf16, tag="w2")
        nc.sync.dma_start(w2_t[:], w2_ap[:])
        b1_t = consts.tile([128, 1], f32, tag="b1")
        nc.sync.dma_start(b1_t[:], b1_ap[:])
        b2_t = consts.tile([128, 1], f32, tag="b2")
        nc.sync.dma_start(b2_t[:], b2_ap[:])
        # all indicator tiles live in SBUF for the whole kernel; the DMA is
        # issued after the first chunks' so it doesn't delay the pipeline
        # start (first stitch consumer is ~10 steps in)
        ind_t = consts.tile([128, NGRP * NCHK * 128], bf16, tag="ind")
        wres_cols = [wres_pool.tile([128, H + 1], bf16, name=f"wres{i}",
                                    tag=f"wres{i}")
                     for i in range(NWCOL)]
        for i in range(NWCOL):
            nc.vector.memset(wres_cols[i][:], 0.0)

        # emit group Gi's stitch right after its last window is flushed
        ready_groups = {}
        for Gi in range(NGRP):
            ready_groups.setdefault(wlo_shared[Gi] + NW_STITCH - 1, []).append(Gi)

        def emit_stitch(Gi):
            st = lg_psum.tile([128, H + 1], f32, tag="lg")
            for k in range(NCHK):
                wc = wlo_shared[Gi] // 2 + k
                nc.tensor.matmul(
                    st[:], lhsT=ind_t[:, (Gi * NCHK + k) * 128:(Gi * NCHK + k + 1) * 128],
                    rhs=wres_cols[wc][:],
                    start=(k == 0), stop=(k == NCHK - 1))
            r = r_pool.tile([128, 1], f32)
            nc.vector.reciprocal(r[:], st[:, H:H + 1])
            ob = ob_pool.tile([128, H], f32)
            nc.vector.tensor_scalar(ob[:], st[:, 0:H], r[:, 0:1], None,
                                    op0=ALU.mult)
            nc.sync.dma_start(out_ap[Gi * 128:(Gi + 1) * 128, :], ob[:])

        # ---- pipeline stages -------------------------------------------
        # step s runs: ht q0,q1 (s) | logits+exp+masked-e (s-1) |
        #              ht q2,q3 (s) | pooling+stitch (s-2)
        # The logit block between the ht halves gives the Act engine time to
        # drain tanh(q0) before the PE reuses its PSUM buffer for q2.
        def s_mlp_a(c):
            """DMA chunk c; ht matmuls+tanh for cols 0:2048; chunk-level eq."""
            xt = xt_pool.tile([128, CHUNK], bf16, name=f"xt{c}", tag="xt")
            nc.sync.dma_start(xt[:], xt_ap[:, c * CHUNK:(c + 1) * CHUNK])
            xn = xn_pool.tile([128, TPC * (H + 1)], bf16, name=f"xn{c}", tag="xn")
            nc.sync.dma_start(
                xn[:], xn_ap[:, c * TPC * (H + 1):(c + 1) * TPC * (H + 1)])
            sg = seg_pool.tile([128, TPC], bf16, name=f"sg{c}", tag="sg")
            nc.sync.dma_start(sg[:], seg_ap[:, c * TPC:(c + 1) * TPC])

            st = dict(xt=xt, xn=xn, hqs=[])
            for quarter in range(2):
                _ht_quarter(c, st, quarter)
            # chunk-level eq: eq[p, t, s] = (sg[p, t] == iota[s])
            eq = eq_pool.tile([128, TPC, MSLOT], bf16, name=f"eq{c}", tag="eq")
            nc.vector.tensor_tensor(
                eq[:], sg[:, :, None].broadcast_to([128, TPC, MSLOT]),
                iota_t[:, None, :].broadcast_to([128, TPC, MSLOT]),
                op=ALU.is_equal)
            st["eq"] = eq
            return st

        def _ht_quarter(c, st, quarter):
            ht = ht_psum.tile([128, 1024], f32, name=f"ht{c}_{quarter}",
                              tag="ht")
            for j in range(2):
                lo = quarter * 1024 + j * 512
                nc.tensor.matmul(ht[:, j * 512:(j + 1) * 512], lhsT=w1_t[:],
                                 rhs=st["xt"][:, lo:lo + 512],
                                 start=True, stop=True)
            hq = h_pool.tile([128, 1024], bf16, name=f"hq{c}_{quarter}",
                             tag="hq")
            nc.scalar.activation(hq[:], ht[:], AF.Tanh, bias=b1_t[:, 0:1])
            st["hqs"].append(hq)

        def s_mlp_b(c, st):
            for quarter in range(2, 4):
                _ht_quarter(c, st, quarter)

        def s_lg(c, st):
            """32 logit matmuls for chunk c; exp."""
            lg = lg_psum.tile([128, TPC], f32, name=f"lg{c}", tag="lg")
            for t in range(TPC):
                hq = st["hqs"][t // 8]
                nc.tensor.matmul(lg[:, t:t + 1],
                                 lhsT=hq[:, (t % 8) * 128:(t % 8 + 1) * 128],
                                 rhs=w2_t[:], start=True, stop=True)
            ee = e_pool.tile([128, TPC], bf16, name=f"ee{c}", tag="ee")
            nc.scalar.activation(ee[:], lg[:], AF.Exp, bias=b2_t[:, 0:1])
            st["ee"] = ee

        def s_memult(c, st):
            """Chunk-level masked-e.  me pools alternate per chunk and the
            pooling depth is even, so a pooling block's pool-level read
            threshold lands on a MULTIPLY finished two steps ago (the
            scheduler serializes pool readers against the latest writer in
            final order, whatever the emission order)."""
            me = me_pool.tile([128, TPC, MSLOT], bf16, name=f"me{c}", tag="me")
            nc.vector.tensor_tensor(
                me[:], st["eq"][:],
                st["ee"][:, :, None].broadcast_to([128, TPC, MSLOT]),
                op=ALU.mult)
            st["me"] = me

        def s_pool_mm(c, st):
            """32 pooling matmuls for chunk c (PE only)."""
            strip = 64 * (c % 2)
            pl = pl_psum.tile([128, H + 1], f32, name=f"pl{c}", tag="pl")
            xn, me = st["xn"], st["me"]
            for t in range(TPC):
                nc.tensor.matmul(
                    pl[strip:strip + MSLOT, :], lhsT=me[:, t, :],
                    rhs=xn[:, t * (H + 1):(t + 1) * (H + 1)],
                    start=(t == 0), stop=(t == TPC - 1),
                    tile_position=(0, strip))
            st["pl"] = pl
            st["strip"] = strip

        def s_pool_flush(c, st):
            """Flush chunk c's window strip to wres; run stitches that became
            ready LAST step (their wres inputs were flushed a step ago, so
            the stitch matmuls never wait on this step's CAST)."""
            nc.scalar.activation(
                wres_cols[c // 2][st["strip"]:st["strip"] + MSLOT, :],
                st["pl"][st["strip"]:st["strip"] + MSLOT, :], AF.Copy)
            for Gi in ready_groups.get(c - 1, ()):
                emit_stitch(Gi)

        # Step order matters: the tile framework serializes a pool's readers
        # against the latest writer EMITTED so far, so the pooling block for
        # chunk c-3 is emitted before this step's masked-e MULTIPLY, and the
        # logit block sits between the two ht halves so tanh(q0) completes
        # before its PSUM buffer is reused.
        # flush (CAST + stitches) leads the NEXT step so neither the Act tail
        # nor the PE stitch matmuls ever wait on freshly-emitted producers
        p1 = p2 = p3 = p4 = p5 = p6 = None
        for c in range(NWIN):
            if p6 is not None:
                s_pool_flush(c - 6, p6)
            if p1 is not None:
                s_lg(c - 1, p1)
                s_memult(c - 1, p1)
            cur = s_mlp_a(c)
            if c == 3:
                nc.sync.dma_start(ind_t[:], ind_ap[:])
            s_mlp_b(c, cur)
            if p5 is not None:
                s_pool_mm(c - 5, p5)
            p6, p5, p4, p3, p2, p1 = p5, p4, p3, p2, p1, cur
        s_lg(NWIN - 1, p1)
        s_memult(NWIN - 1, p1)
        s_pool_flush(NWIN - 6, p6)
        for cc, st in ((NWIN - 5, p5), (NWIN - 4, p4), (NWIN - 3, p3),
                       (NWIN - 2, p2), (NWIN - 1, p1)):
            s_pool_mm(cc, st)
            s_pool_flush(cc, st)
        for Gi in ready_groups.get(NWIN - 1, ()):
            emit_stitch(Gi)

    nc.compile()
    return nc


def kernel(x, batch, W1, b1, W2, b2):
    global LAST_EXEC_NS
    import os
    from concourse.bass_utils import run_bass_kernel_spmd

    x = np.asarray(x)
    batch = np.asarray(batch)
    shared, cores = _preprocess(x, batch, W1, b1, W2, b2, N_GRAPHS)

    key = (shared["NWIN"], shared["NGRP"], tuple(shared["wlo_shared"]))
    nc = _PROGRAM_CACHE.get(key)
    if nc is None:
        nc = _build_program(shared["NWIN"], shared["NGRP"], shared["wlo_shared"])
        _PROGRAM_CACHE[key] = nc

    in_maps = []
    for d in cores:
        in_maps.append({
            "xT": d["xt_img"], "xn": d["xn_img"], "seg": d["seg_img"],
            "iota": shared["iota"], "w1b": shared["w1b"], "w2b": shared["w2b"],
            "b1c": shared["b1c"], "b2c": shared["b2c"], "ind": d["ind_img"],
        })
    trace = os.environ.get("ATTNPOOL_TRACE", "0") == "1"
    res = run_bass_kernel_spmd(nc, in_maps, core_ids=list(range(N_CORES)),
                               trace=trace)
    if res.exec_time_ns is not None:
        LAST_EXEC_NS = res.exec_time_ns

    out = np.zeros((N_GRAPHS, H), dtype=np.float32)
    gsplit = shared["gsplit"]
    for c, d in enumerate(cores):
        out[gsplit[c]:gsplit[c + 1]] = res.results[c]["out"][:d["gc"]]
    out[shared["counts"] == 0] = 0.0
    return out
